# revision 11
# baseline (speedup 1.0000x reference)
# LSTM (embedding -> single-layer LSTM -> linear head) on Trainium2.
#
# Sharding: data-parallel over batch, B=64 -> NCORES cores x bloc rows.
#
# Per-core layout ("feature-major"): the 4*H=2048 gate columns are permuted
# host-side into 4 lanes of 512, lane j = [f|i|g|o] x 128 for H-slice j
# (128 H values). Lane j is computed into PSUM partition group 32j..32j+bloc
# by a column-tiled matmul (tile_position=(0, 32j)), so:
#   - every per-element tensor (i,f,g,o,c,h) lives on a [128, 128] tile
#     (partition = 32j + b, free = H-within-slice) -> all DVE/ACT ops are
#     full-partition, FD=128, and share partition bases (no rebasing).
#   - h's [128, 128] tile is square: ONE PE transpose per step yields the
#     stationary operand for all 4 K-chunks of the next step's matmuls
#     (hT[:, 32k:32k+bloc] = lhsT for K-chunk k).
# x_proj is precomputed per 128-token chunk (indirect-DMA gather -> PE
# transpose -> GEMM + bias via ones-row matmul) and injected into PSUM with
# identity-column matmuls (lhsT = I[:, rp:rp+32], K=128) -- the same
# 128x32 column-tiled PE mode as the recurrence, so no PE mode switches
# inside a step. Column-tiled matmuls use float16 (1 cyc/row, 10-bit
# mantissa; the ISA forbids fp32r off the full col_grp=0xf array). The
# full-width producer GEMM runs in f32r (e8m11), rounded on-device by its
# producer copies (walrus requirement).
import numpy as np

_VOCAB, _EMB, _HID = 50257, 512, 512
_B, _T = 64, 1024
_NCORES = 8
_BLOC = _B // _NCORES


def _build(T, vocab, bloc=_BLOC):
    """Build the per-core Bass program. Returns (nc, input_names, out_name)."""
    import concourse.bass as bass
    import concourse.mybir as mybir
    import concourse.tile as tile
    from concourse import bacc
    from concourse.masks import make_identity

    f32 = mybir.dt.float32
    f32r = mybir.dt.float32r
    f16 = mybir.dt.float16
    i32 = mybir.dt.int32
    SIG = mybir.ActivationFunctionType.Sigmoid
    TANH = mybir.ActivationFunctionType.Tanh
    MUL = mybir.AluOpType.mult
    ADD = mybir.AluOpType.add

    TPC = 128 // bloc            # timesteps per 128-token chunk
    assert T % TPC == 0
    NCH = T // TPC               # number of chunks
    H4 = 4 * _HID

    nc = bacc.Bacc("TRN2", target_bir_lowering=False, debug=False)

    # ---- DRAM I/O (per-core) ----
    emb_d = nc.dram_tensor("emb", [vocab, _EMB], f32, kind="ExternalInput")
    xidx_d = nc.dram_tensor("xidx", [bloc * T, 1], i32, kind="ExternalInput")
    whhT_d = nc.dram_tensor("whhT", [128, 4, H4], f16, kind="ExternalInput")
    wihT_d = nc.dram_tensor("wihT", [128, 4, H4], f32, kind="ExternalInput")
    bias_d = nc.dram_tensor("bias", [1, H4], f32, kind="ExternalInput")
    woutT_d = nc.dram_tensor("woutT", [128, 4, 2], f16, kind="ExternalInput")
    bout_d = nc.dram_tensor("bout", [1, 2], f16, kind="ExternalInput")
    out_d = nc.dram_tensor("out", [bloc, 2], f32, kind="ExternalOutput")

    with tile.TileContext(nc) as tc:
        with (
            tc.tile_pool(name="const", bufs=1) as pc,
            tc.tile_pool(name="stage", bufs=1) as pstg,
            tc.tile_pool(name="cst", bufs=2) as pcs,
            tc.tile_pool(name="hst", bufs=2) as ph,
            tc.tile_pool(name="hTt", bufs=2) as phT,
            tc.tile_pool(name="act", bufs=2) as pact,
            tc.tile_pool(name="tmp", bufs=2) as ptmp,
            tc.tile_pool(name="idx", bufs=2) as pidx,
            tc.tile_pool(name="emb", bufs=2) as pe,
            tc.tile_pool(name="eT", bufs=2) as peT,
            tc.tile_pool(name="xb", bufs=2) as pxb,
            tc.tile_pool(name="ps_g", bufs=2, space="PSUM") as pp_g,
            tc.tile_pool(name="ps_t", bufs=2, space="PSUM") as pp_t,
            tc.tile_pool(name="ps_p", bufs=2, space="PSUM") as pp_p,
        ):
            # ---- weights: DMA fp32 staging -> round into f32r tiles ----
            whhT = pc.tile([128, 4, H4], f16)
            nc.sync.dma_start(whhT[:], whhT_d.ap())
            wihT = pc.tile([128, 4, H4], f32r)
            stg = pstg.tile([128, 4, H4], f32, tag="stg")
            nc.sync.dma_start(stg[:], wihT_d.ap())
            nc.vector.tensor_copy(wihT[:], stg[:])

            bias_t = pc.tile([1, H4], f32r)
            stg_b = pstg.tile([1, H4], f32, tag="stg_b")
            nc.sync.dma_start(stg_b[:], bias_d.ap())
            nc.vector.tensor_copy(bias_t[:], stg_b[:])

            woutT = pc.tile([128, 4, 2], f16)
            nc.sync.dma_start(woutT[:], woutT_d.ap())
            bout_t = pc.tile([1, 2], f16)
            nc.sync.dma_start(bout_t[:], bout_d.ap())

            ident = pc.tile([128, 128], f32)      # for PE transposes
            make_identity(nc, ident[:])
            # extended identity for injects: cols 128..160 wrap back to 0..32
            # so an M=32 inject near the chunk end stays in range. Rows 8..31
            # of each partition group get filler tokens -> every PSUM/SBUF
            # partition is written (no uninitialized/NaN lanes anywhere).
            identx = pc.tile([128, 160], f16)
            nc.vector.tensor_copy(identx[:, 0:128], ident[:])
            nc.vector.tensor_copy(identx[:, 128:160], ident[:, 0:32])
            ones_f = pc.tile([1, 128], f32)
            nc.vector.memset(ones_f[:], 1.0)
            ones_r = pc.tile([1, 128], f32r)
            nc.vector.tensor_copy(ones_r[:], ones_f[:])
            ones_h = pc.tile([1, 128], f16)
            nc.vector.tensor_copy(ones_h[:], ones_f[:])
            zero_b = pc.tile([128, 1], f32)       # explicit ACT bias
            nc.vector.memset(zero_b[:], 0.0)

            c0 = pcs.tile([128, 128], f32)
            nc.vector.memset(c0[:], 0.0)

            # ---- x_proj producer, split into phases so pieces can be
            # emitted between recurrence steps (engines are strict FIFO:
            # a producer op queued ahead of step ops whose inputs aren't
            # ready yet would head-of-line block the step) ----
            def produce_gather(cb):
                idx_t = pidx.tile([128, 1], i32)
                nc.sync.dma_start(idx_t[:], xidx_d.ap()[cb * 128:(cb + 1) * 128, :])
                e_t = pe.tile([128, _EMB], f32)
                nc.gpsimd.indirect_dma_start(
                    out=e_t[:],
                    out_offset=None,
                    in_=emb_d.ap(),
                    in_offset=bass.IndirectOffsetOnAxis(ap=idx_t[:, :1], axis=0),
                )
                return e_t

            def produce_eT(e_t):
                ps_e = pp_p.tile([128, 4, 128], f32, space="PSUM", tag="prod")
                for k in range(4):
                    nc.tensor.transpose(ps_e[:, k, :], e_t[:, k * 128:(k + 1) * 128], ident[:])
                eT = peT.tile([128, 4, 128], f32r)
                nc.scalar.copy(eT[:], ps_e[:])
                return eT

            def produce_lane(eT, xb, j):
                ps_x = pp_p.tile([128, 512], f32, space="PSUM", tag="prod")
                for k in range(4):
                    nc.tensor.matmul(
                        ps_x[:], eT[:, k, :], wihT[:, k, j * 512:(j + 1) * 512],
                        start=(k == 0), stop=False,
                    )
                nc.tensor.matmul(
                    ps_x[:], ones_r[0:1, 0:128], bias_t[0:1, j * 512:(j + 1) * 512],
                    start=False, stop=True,
                )
                if j % 2 == 0:
                    nc.scalar.copy(xb[:, j, :], ps_x[:])
                else:
                    nc.vector.tensor_copy(xb[:, j, :], ps_x[:])

            # ---- inject x_proj for step t into a fresh psum gate tile ----
            def inject(t, xb):
                rp = (t % TPC) * bloc
                psg = pp_g.tile([128, 512], f32, space="PSUM")
                for j in range(4):
                    nc.tensor.matmul(
                        psg[32 * j:32 * j + 32, :],
                        identx[:, rp:rp + 32],
                        xb[:, j, :],
                        start=True, stop=(t == 0),
                        tile_position=(0, 32 * j),
                        skip_group_check=True,
                    )
                return psg

            # ---- one recurrence step ----
            def step_mm(psg, hT):
                # gates += h @ w_hh.T  (4 col-tiled lanes x 4 K-chunks)
                for k in range(4):
                    for j in range(4):
                        nc.tensor.matmul(
                            psg[32 * j:32 * j + 32, :],
                            hT[:, 32 * k:32 * k + 32],
                            whhT[:, k, j * 512:(j + 1) * 512],
                            start=False, stop=(k == 3),
                            tile_position=(0, 32 * j),
                            skip_group_check=True,
                        )

            def step_cell(psg, c_prev):
                # lanes hold [f | i | g | o] x 128 along free dim
                act = pact.tile([128, 512], f32r)
                nc.scalar.activation(act[:, 0:256], psg[:, 0:256], SIG,
                                     bias=zero_b[:, 0:1])
                nc.scalar.activation(act[:, 256:384], psg[:, 256:384], TANH,
                                     bias=zero_b[:, 0:1])
                nc.scalar.activation(act[:, 384:512], psg[:, 384:512], SIG,
                                     bias=zero_b[:, 0:1])
                fc = ptmp.tile([128, 128], f32, tag="fc")
                nc.vector.tensor_tensor(fc[:], act[:, 0:128], c_prev[:], MUL)
                ig = ptmp.tile([128, 128], f32, tag="ig")
                nc.vector.tensor_tensor(ig[:], act[:, 128:256], act[:, 256:384], MUL)
                c_new = pcs.tile([128, 128], f32)
                nc.vector.tensor_tensor(c_new[:], fc[:], ig[:], ADD)
                thc = ptmp.tile([128, 128], f32r, tag="thc")
                nc.scalar.activation(thc[:], c_new[:], TANH, bias=zero_b[:, 0:1])
                h = ph.tile([128, 128], f32)
                nc.vector.tensor_tensor(h[:], act[:, 384:512], thc[:], MUL)
                ps_t = pp_t.tile([128, 512], f32, space="PSUM")
                nc.tensor.transpose(ps_t[:, 0:128], h[:], ident[:])
                hT = phT.tile([128, 128], f16)
                nc.vector.tensor_copy(hT[:], ps_t[:, 0:128])
                return hT, c_new

            # ---- main program ----
            # chunk 0 producer runs compactly up front
            e_t = produce_gather(0)
            eT = produce_eT(e_t)
            xb = pxb.tile([128, 4, 512], f16)
            for j in range(4):
                produce_lane(eT, xb, j)
            psg = inject(0, xb)
            hT = None
            c_sb = c0
            # producer phases for chunk cb+1 are emitted after the cell ops
            # of selected steps of chunk cb (spread; ready before needed)
            assert TPC >= 11, "producer spreading assumes bloc <= 8"
            r_eT = 1
            lane_r = {3: 0, 5: 1, 7: 2, 9: 3}
            for cb in range(NCH):
                if cb + 1 < NCH:
                    e_next = produce_gather(cb + 1)
                    xb_next = pxb.tile([128, 4, 512], f16)
                else:
                    e_next = xb_next = None
                for r in range(TPC):
                    t = cb * TPC + r
                    if hT is not None:
                        step_mm(psg, hT)
                    # inject t+1 early: runs on PE during this step's ACT/DVE
                    if t + 1 < T:
                        xb_n = xb if r + 1 < TPC else xb_next
                        psg_next = inject(t + 1, xb_n)
                    else:
                        psg_next = None
                    hT, c_sb = step_cell(psg, c_sb)
                    if e_next is not None:
                        if r == r_eT:
                            eT_next = produce_eT(e_next)
                        elif r in lane_r:
                            produce_lane(eT_next, xb_next, lane_r[r])
                    psg = psg_next
                xb = xb_next

            # ---- output head: out = h_last @ w_out.T + b_out ----
            ps_o = pp_t.tile([128, 512], f32, space="PSUM")
            for k in range(4):
                nc.tensor.matmul(
                    ps_o[0:bloc, 0:2], hT[:, 32 * k:32 * k + bloc], woutT[:, k, :],
                    start=(k == 0), stop=False,
                )
            nc.tensor.matmul(
                ps_o[0:bloc, 0:2], ones_h[0:1, 0:bloc], bout_t[0:1, :],
                start=False, stop=True,
            )
            o_sb = pc.tile([bloc, 2], f32)
            nc.vector.tensor_copy(o_sb[:], ps_o[0:bloc, 0:2])
            nc.sync.dma_start(out_d.ap(), o_sb[:])

    nc.compile()
    in_names = ["emb", "xidx", "whhT", "wihT", "bias", "woutT", "bout"]
    return nc, in_names, "out"


def _perm_gate_rows(w):
    """Rows gate-major [i,f,g,o]x512 -> lane-major: new row j*512 + s*128 + m
    (s in [f,i,g,o] order) = old row og(s)*512 + j*128 + m."""
    w4 = np.asarray(w, np.float32).reshape(4, 4, 128, -1)   # [gate, j, m, X]
    t = w4[[1, 0, 2, 3]]                                     # [s, j, m, X]
    return np.ascontiguousarray(t.transpose(1, 0, 2, 3)).reshape(2048, -1)


def _prep_host(x, emb, w_ih, w_hh, b_ih, b_hh, w_out, b_out, bloc, ncores):
    """Host-side reshapes: gate-lane permutation, weight transposes into
    [128, 4, *] K-major tiles, per-core t-major index lists."""
    w_ih_p = _perm_gate_rows(w_ih)
    w_hh_p = _perm_gate_rows(w_hh)
    bias_p = _perm_gate_rows(
        (np.asarray(b_ih, np.float32) + np.asarray(b_hh, np.float32))[:, None]
    )[:, 0]

    # wT[p, k, n] = w_p[n, 128k + p]
    def to_kt(w_p):
        return np.ascontiguousarray(w_p.T.reshape(4, 128, w_p.shape[0]).transpose(1, 0, 2))

    whhT = to_kt(w_hh_p).astype(np.float16)
    wihT = to_kt(w_ih_p)
    woutT = np.ascontiguousarray(
        np.asarray(w_out, np.float32).T.reshape(4, 128, 2).transpose(1, 0, 2)
    ).astype(np.float16)

    emb_c = np.ascontiguousarray(np.asarray(emb, np.float32))
    bias_c = np.ascontiguousarray(bias_p[None, :])
    bout_c = np.ascontiguousarray(np.asarray(b_out, np.float16)[None, :])

    x = np.asarray(x)
    B, T = x.shape
    in_maps = []
    for c in range(ncores):
        xs = x[c * bloc:(c + 1) * bloc, :]          # [bloc, T]
        xidx = np.ascontiguousarray(xs.T.reshape(bloc * T, 1)).astype(np.int32)
        in_maps.append({
            "emb": emb_c,
            "xidx": xidx,
            "whhT": whhT,
            "wihT": wihT,
            "bias": bias_c,
            "woutT": woutT,
            "bout": bout_c,
        })
    return in_maps


_CACHE = {}


def kernel(x, emb, w_ih, w_hh, b_ih, b_hh, w_out, b_out):
    from concourse.bass_utils import run_bass_kernel_spmd

    x = np.asarray(x)
    B, T = x.shape
    ncores = _NCORES
    bloc = B // ncores
    vocab = emb.shape[0]

    key = (T, vocab, bloc)
    if key not in _CACHE:
        _CACHE[key] = _build(T, vocab, bloc)
    nc, in_names, out_name = _CACHE[key]

    in_maps = _prep_host(x, emb, w_ih, w_hh, b_ih, b_hh, w_out, b_out, bloc, ncores)
    res = run_bass_kernel_spmd(nc, in_maps, core_ids=list(range(ncores)))
    out = np.concatenate([r[out_name] for r in res.results], axis=0)  # [B, 2]
    return out


if __name__ == "__main__":
    _build(_T, _VOCAB, _BLOC)
    print("build ok")
